# revision 8
# baseline (speedup 1.0000x reference)
"""2D Haar DWT (single level, reflect-pad) Trainium2 Bass kernel.

Input  x: (16, 64, 256, 256) fp32
Output y: (16, 256, 129, 129) fp32, channel layout [ll(64C), lh, hl, hh].

Strategy: pure data parallel over 8 NeuronCores; each core gets 128 of the
1024 (batch, channel) images, one image per SBUF partition.  All butterfly
arithmetic runs along the free dimension:

  stage 1 (column butterfly, VectorE):
      lo[r, j] = 0.5*x[r, 2j-1] + 0.5*x[r, 2j]      (edges j=0, j=W/2 special)
      hi[r, j] = 0.5*x[r, 2j]   - 0.5*x[r, 2j-1]
  stage 2 (row butterfly, VectorE):
      ll[i] = lo[2i-1] + lo[2i]   lh[i] = hi[2i-1] + hi[2i]
      hl[i] = lo[2i]   - lo[2i-1] hh[i] = hi[2i]   - hi[2i-1]
      (edges i=0, i=H/2 special; the 0.5 prescale runs on ScalarE)

Output rows are produced in chunks of KO so DMAs stay in the multi-MB range.
"""

import numpy as np

import concourse.mybir as mybir
import concourse.tile as tile
from concourse import bacc
from concourse.bass_utils import run_bass_kernel_spmd

N_CORES = 8
F32 = mybir.dt.float32


def _emit_dwt(tc, xa, ya, ko, out_engine="scalar", bufs=2):
    nc = tc.nc
    n, h, w = xa.shape
    assert n == nc.NUM_PARTITIONS
    hh = h // 2
    wo = w // 2 + 1
    assert ko < hh

    # (i0, n_out, r0, n_rows, first, last)
    chunks = [(0, ko, 0, 2 * ko - 1, True, False)]
    i0 = ko
    while i0 < hh:
        ni = min(ko, hh - i0)
        r0 = 2 * i0 - 1
        if i0 + ni == hh:
            chunks.append((i0, ni + 1, r0, h - r0, False, True))
        else:
            chunks.append((i0, ni, r0, 2 * ni, False, False))
        i0 += ni

    with (
        tc.tile_pool(name="px", bufs=bufs) as px,
        tc.tile_pool(name="plh", bufs=2) as plh,
        tc.tile_pool(name="py", bufs=bufs) as py,
    ):
        for i0, nout, r0, nr, first, last in chunks:
            xt = px.tile([n, nr, w], F32, tag="xt")
            nc.sync.dma_start(out=xt[:], in_=xa[:, r0 : r0 + nr, :])
            nc.scalar.mul(xt[:], xt[:], 0.5)

            lo = plh.tile([n, nr, wo], F32, tag="lo")
            hi = plh.tile([n, nr, wo], F32, tag="hi")
            a = xt[:, :, 1 : w - 1 : 2]
            b = xt[:, :, 2:w:2]
            nc.vector.tensor_add(out=lo[:, :, 1 : wo - 1], in0=a, in1=b)
            nc.vector.tensor_sub(out=hi[:, :, 1 : wo - 1], in0=b, in1=a)
            # j=0 edge: cols (1, 0) of x -> lo = x0+x1, hi = x0-x1
            nc.vector.tensor_add(out=lo[:, :, 0:1], in0=xt[:, :, 0:1], in1=xt[:, :, 1:2])
            nc.vector.tensor_sub(out=hi[:, :, 0:1], in0=xt[:, :, 0:1], in1=xt[:, :, 1:2])
            # j=W/2 edge: cols (W-1, W-2) -> lo = x[W-2]+x[W-1], hi = x[W-2]-x[W-1]
            nc.vector.tensor_add(
                out=lo[:, :, wo - 1 : wo], in0=xt[:, :, w - 2 : w - 1], in1=xt[:, :, w - 1 : w]
            )
            nc.vector.tensor_sub(
                out=hi[:, :, wo - 1 : wo], in0=xt[:, :, w - 2 : w - 1], in1=xt[:, :, w - 1 : w]
            )

            yt = py.tile([n, 4, nout, wo], F32, tag="yt")
            if first:
                ts, bs, nint, oo = 1, 2, nout - 1, 1
                specials = [(1, 0, 0)]
            elif last:
                ts, bs, nint, oo = 0, 1, nout - 1, 0
                specials = [(nr - 1, nr - 2, nout - 1)]
            else:
                ts, bs, nint, oo = 0, 1, nout, 0
                specials = []

            for sb, src, sub in ((0, lo, False), (1, hi, False), (2, lo, True), (3, hi, True)):
                tv = src[:, ts : ts + 2 * nint - 1 : 2, :]
                bv = src[:, bs : bs + 2 * nint - 1 : 2, :]
                ov = yt[:, sb, oo : oo + nint, :]
                if sub:
                    nc.vector.tensor_sub(out=ov, in0=bv, in1=tv)
                else:
                    nc.vector.tensor_add(out=ov, in0=tv, in1=bv)
                for tt, bb, orow in specials:
                    tv1 = src[:, tt : tt + 1, :]
                    bv1 = src[:, bb : bb + 1, :]
                    ov1 = yt[:, sb, orow : orow + 1, :]
                    if sub:
                        nc.vector.tensor_sub(out=ov1, in0=bv1, in1=tv1)
                    else:
                        nc.vector.tensor_add(out=ov1, in0=tv1, in1=bv1)

            out_eng = {"scalar": nc.scalar, "sync": nc.sync, "gpsimd": nc.gpsimd}[out_engine]
            out_eng.dma_start(out=ya[:, :, i0 : i0 + nout, :], in_=yt[:])


def build_dwt_bass(n_img, h, w, ko=14, reps=1, out_engine="scalar", bufs=2):
    nc = bacc.Bacc("TRN2", target_bir_lowering=False, debug=False)
    x = nc.dram_tensor("x", [n_img, h, w], F32, kind="ExternalInput")
    y = nc.dram_tensor("y", [n_img, 4, h // 2 + 1, w // 2 + 1], F32, kind="ExternalOutput")
    with tile.TileContext(nc) as tc:
        for _ in range(reps):
            _emit_dwt(tc, x.ap(), y.ap(), ko, out_engine=out_engine, bufs=bufs)
    nc.compile()
    return nc


_NC_CACHE = {}


def _get_nc(n_img, h, w):
    key = (n_img, h, w)
    nc = _NC_CACHE.get(key)
    if nc is None:
        nc = _NC_CACHE[key] = build_dwt_bass(n_img, h, w)
    return nc


def kernel(x, _results_hook=None):
    x = np.ascontiguousarray(np.asarray(x), dtype=np.float32)
    b, c, h, w = x.shape
    n_total = b * c
    n_img = n_total // N_CORES
    nc = _get_nc(n_img, h, w)
    xf = x.reshape(n_total, h, w)
    in_maps = [{"x": xf[i * n_img : (i + 1) * n_img]} for i in range(N_CORES)]
    r = run_bass_kernel_spmd(nc, in_maps, list(range(N_CORES)))
    if _results_hook is not None:
        _results_hook(r)
    ho, wo = h // 2 + 1, w // 2 + 1
    out = np.concatenate([m["y"] for m in r.results], axis=0)
    out = out.reshape(b, c, 4, ho, wo).transpose(0, 2, 1, 3, 4).reshape(b, 4 * c, ho, wo)
    return np.ascontiguousarray(out)


# revision 12
# speedup vs baseline: 1.0238x; 1.0238x over previous
"""2D Haar DWT (single level, reflect-pad) Trainium2 Bass kernel.

Input  x: (16, 64, 256, 256) fp32
Output y: (16, 256, 129, 129) fp32, channel layout [ll(64C), lh, hl, hh].

Strategy: pure data parallel over 8 NeuronCores; each core gets 128 of the
1024 (batch, channel) images, one image per SBUF partition.  All butterfly
arithmetic runs along the free dimension:

  stage 1 (column butterfly, VectorE):
      lo[r, j] = 0.5*x[r, 2j-1] + 0.5*x[r, 2j]      (edges j=0, j=W/2 special)
      hi[r, j] = 0.5*x[r, 2j]   - 0.5*x[r, 2j-1]
  stage 2 (row butterfly, VectorE):
      ll[i] = lo[2i-1] + lo[2i]   lh[i] = hi[2i-1] + hi[2i]
      hl[i] = lo[2i]   - lo[2i-1] hh[i] = hi[2i]   - hi[2i-1]
      (edges i=0, i=H/2 special; the 0.5 prescale runs on ScalarE)

Output rows are produced in chunks of KO so DMAs stay in the multi-MB range.
"""

import numpy as np

import concourse.mybir as mybir
import concourse.tile as tile
from concourse import bacc
from concourse.bass_utils import run_bass_kernel_spmd

N_CORES = 8
F32 = mybir.dt.float32


def _emit_dwt(tc, xa, ya, ko, out_engine="scalar", bufs=2):
    nc = tc.nc
    n, h, w = xa.shape
    assert n == nc.NUM_PARTITIONS
    hh = h // 2
    wo = w // 2 + 1
    assert ko < hh

    # (i0, n_out, r0, n_rows, first, last)
    chunks = [(0, ko, 0, 2 * ko - 1, True, False)]
    i0 = ko
    while i0 < hh:
        ni = min(ko, hh - i0)
        r0 = 2 * i0 - 1
        if i0 + ni == hh:
            chunks.append((i0, ni + 1, r0, h - r0, False, True))
        else:
            chunks.append((i0, ni, r0, 2 * ni, False, False))
        i0 += ni

    with (
        tc.tile_pool(name="px", bufs=bufs) as px,
        tc.tile_pool(name="plh", bufs=2) as plh,
        tc.tile_pool(name="py", bufs=bufs) as py,
    ):
        for i0, nout, r0, nr, first, last in chunks:
            xt = px.tile([n, nr, w], F32, tag="xt")
            nc.sync.dma_start(out=xt[:], in_=xa[:, r0 : r0 + nr, :])
            nc.scalar.mul(xt[:], xt[:], 0.5)

            lo = plh.tile([n, nr, wo], F32, tag="lo")
            hi = plh.tile([n, nr, wo], F32, tag="hi")
            a = xt[:, :, 1 : w - 1 : 2]
            b = xt[:, :, 2:w:2]
            nc.vector.tensor_add(out=lo[:, :, 1 : wo - 1], in0=a, in1=b)
            nc.vector.tensor_sub(out=hi[:, :, 1 : wo - 1], in0=b, in1=a)
            # j=0 edge: cols (1, 0) of x -> lo = x0+x1, hi = x0-x1
            nc.vector.tensor_add(out=lo[:, :, 0:1], in0=xt[:, :, 0:1], in1=xt[:, :, 1:2])
            nc.vector.tensor_sub(out=hi[:, :, 0:1], in0=xt[:, :, 0:1], in1=xt[:, :, 1:2])
            # j=W/2 edge: cols (W-1, W-2) -> lo = x[W-2]+x[W-1], hi = x[W-2]-x[W-1]
            nc.vector.tensor_add(
                out=lo[:, :, wo - 1 : wo], in0=xt[:, :, w - 2 : w - 1], in1=xt[:, :, w - 1 : w]
            )
            nc.vector.tensor_sub(
                out=hi[:, :, wo - 1 : wo], in0=xt[:, :, w - 2 : w - 1], in1=xt[:, :, w - 1 : w]
            )

            yt = py.tile([n, 4, nout, wo], F32, tag="yt")
            if first:
                ts, bs, nint, oo = 1, 2, nout - 1, 1
                specials = [(1, 0, 0)]
            elif last:
                ts, bs, nint, oo = 0, 1, nout - 1, 0
                specials = [(nr - 1, nr - 2, nout - 1)]
            else:
                ts, bs, nint, oo = 0, 1, nout, 0
                specials = []

            for sb, src, sub in ((0, lo, False), (1, hi, False), (2, lo, True), (3, hi, True)):
                tv = src[:, ts : ts + 2 * nint - 1 : 2, :]
                bv = src[:, bs : bs + 2 * nint - 1 : 2, :]
                ov = yt[:, sb, oo : oo + nint, :]
                if sub:
                    nc.vector.tensor_sub(out=ov, in0=bv, in1=tv)
                else:
                    nc.vector.tensor_add(out=ov, in0=tv, in1=bv)
                for tt, bb, orow in specials:
                    tv1 = src[:, tt : tt + 1, :]
                    bv1 = src[:, bb : bb + 1, :]
                    ov1 = yt[:, sb, orow : orow + 1, :]
                    if sub:
                        nc.vector.tensor_sub(out=ov1, in0=bv1, in1=tv1)
                    else:
                        nc.vector.tensor_add(out=ov1, in0=tv1, in1=bv1)

            out_eng = {"scalar": nc.scalar, "sync": nc.sync, "gpsimd": nc.gpsimd}[out_engine]
            out_eng.dma_start(out=ya[:, :, i0 : i0 + nout, :], in_=yt[:])


def _emit_dwt_v2(tc, xa, ya, ko, bufs=2):
    """Overlap-tuned variant: split in-DMA/prescale/stage1 into row halves,
    merge the two stage-1 edge columns into one strided op, and split the
    out-DMA across the scalar and gpsimd DGE rings."""
    nc = tc.nc
    n, h, w = xa.shape
    assert n == nc.NUM_PARTITIONS
    hh = h // 2
    wo = w // 2 + 1
    assert ko < hh

    chunks = [(0, ko, 0, 2 * ko - 1, True, False)]
    i0 = ko
    while i0 < hh:
        ni = min(ko, hh - i0)
        r0 = 2 * i0 - 1
        if i0 + ni == hh:
            chunks.append((i0, ni + 1, r0, h - r0, False, True))
        else:
            chunks.append((i0, ni, r0, 2 * ni, False, False))
        i0 += ni

    with (
        tc.tile_pool(name="px", bufs=bufs) as px,
        tc.tile_pool(name="plh", bufs=2) as plh,
        tc.tile_pool(name="py", bufs=bufs) as py,
    ):
        for i0, nout, r0, nr, first, last in chunks:
            xt = px.tile([n, nr, w], F32, tag="xt")
            lo = plh.tile([n, nr, wo], F32, tag="lo")
            hi = plh.tile([n, nr, wo], F32, tag="hi")
            h1 = nr // 2
            for lo_r, hi_r in ((0, h1), (h1, nr)):
                xh = xt[:, lo_r:hi_r, :]
                nc.sync.dma_start(out=xh, in_=xa[:, r0 + lo_r : r0 + hi_r, :])
                nc.scalar.mul(xh, xh, 0.5)
                a = xt[:, lo_r:hi_r, 1 : w - 1 : 2]
                b = xt[:, lo_r:hi_r, 2:w:2]
                nc.vector.tensor_add(out=lo[:, lo_r:hi_r, 1 : wo - 1], in0=a, in1=b)
                nc.vector.tensor_sub(out=hi[:, lo_r:hi_r, 1 : wo - 1], in0=b, in1=a)
                # merged edge op: cols {0,128} of lo/hi from x cols {0,254},{1,255}
                e0 = xt[:, lo_r:hi_r, 0 : w - 1 : w - 2]
                e1 = xt[:, lo_r:hi_r, 1:w : w - 2]
                nc.vector.tensor_add(out=lo[:, lo_r:hi_r, 0 : wo : wo - 1], in0=e0, in1=e1)
                nc.vector.tensor_sub(out=hi[:, lo_r:hi_r, 0 : wo : wo - 1], in0=e0, in1=e1)

            yt = py.tile([n, 4, nout, wo], F32, tag="yt")
            if first:
                ts, bs, nint, oo = 1, 2, nout - 1, 1
                specials = [(1, 0, 0)]
            elif last:
                ts, bs, nint, oo = 0, 1, nout - 1, 0
                specials = [(nr - 1, nr - 2, nout - 1)]
            else:
                ts, bs, nint, oo = 0, 1, nout, 0
                specials = []

            for sb, src, sub in ((0, lo, False), (1, hi, False), (2, lo, True), (3, hi, True)):
                tv = src[:, ts : ts + 2 * nint - 1 : 2, :]
                bv = src[:, bs : bs + 2 * nint - 1 : 2, :]
                ov = yt[:, sb, oo : oo + nint, :]
                if sub:
                    nc.vector.tensor_sub(out=ov, in0=bv, in1=tv)
                else:
                    nc.vector.tensor_add(out=ov, in0=tv, in1=bv)
                for tt, bb, orow in specials:
                    tv1 = src[:, tt : tt + 1, :]
                    bv1 = src[:, bb : bb + 1, :]
                    ov1 = yt[:, sb, orow : orow + 1, :]
                    if sub:
                        nc.vector.tensor_sub(out=ov1, in0=bv1, in1=tv1)
                    else:
                        nc.vector.tensor_add(out=ov1, in0=tv1, in1=bv1)
                if sb == 1:
                    nc.scalar.dma_start(out=ya[:, 0:2, i0 : i0 + nout, :], in_=yt[:, 0:2, :, :])
            nc.gpsimd.dma_start(out=ya[:, 2:4, i0 : i0 + nout, :], in_=yt[:, 2:4, :, :])


def _emit_dwt_v3(tc, xa, ya, ko=12, xbufs=3, ramp=True):
    """Deeper DMA queue variant: 3 input buffers (so two in-DMAs can be in
    flight beyond the chunk being computed), lo/hi fused into one tile to fit
    SBUF, optional small ramp-up chunks to shorten pipeline fill."""
    nc = tc.nc
    n, h, w = xa.shape
    assert n == nc.NUM_PARTITIONS
    hh = h // 2
    wo = w // 2 + 1

    # chunk output-row counts: optional small first chunks, then ko-sized,
    # remainder merged into the final chunk together with row i=hh.
    sizes = []
    rem = hh  # interior outputs 0..hh-1; i=hh rides with the last chunk
    if ramp and hh > 2 * ko:
        for s in (max(2, ko // 4), max(3, ko // 2)):
            sizes.append(s)
            rem -= s
    while rem > ko + 1:
        sizes.append(ko)
        rem -= ko
    sizes.append(rem)

    chunks = []
    i0 = 0
    for idx, sz in enumerate(sizes):
        first = idx == 0
        last = idx == len(sizes) - 1
        if first:
            chunks.append((0, sz, 0, 2 * sz - 1, True, False))
        elif last:
            r0 = 2 * i0 - 1
            chunks.append((i0, sz + 1, r0, h - r0, False, True))
        else:
            chunks.append((i0, sz, 2 * i0 - 1, 2 * sz, False, False))
        i0 += sz
    assert i0 == hh

    with (
        tc.tile_pool(name="px", bufs=xbufs) as px,
        tc.tile_pool(name="plh", bufs=2) as plh,
        tc.tile_pool(name="py", bufs=2) as py,
    ):
        for i0, nout, r0, nr, first, last in chunks:
            xt = px.tile([n, nr, w], F32, tag="xt")
            nc.sync.dma_start(out=xt[:], in_=xa[:, r0 : r0 + nr, :])
            nc.scalar.mul(xt[:], xt[:], 0.5)

            lh = plh.tile([n, 2, nr, wo], F32, tag="lh")
            lo = lh[:, 0]
            hi = lh[:, 1]
            a = xt[:, :, 1 : w - 1 : 2]
            b = xt[:, :, 2:w:2]
            nc.vector.tensor_add(out=lo[:, :, 1 : wo - 1], in0=a, in1=b)
            nc.vector.tensor_sub(out=hi[:, :, 1 : wo - 1], in0=b, in1=a)
            e0 = xt[:, :, 0 : w - 1 : w - 2]
            e1 = xt[:, :, 1:w : w - 2]
            nc.vector.tensor_add(out=lo[:, :, 0 : wo : wo - 1], in0=e0, in1=e1)
            nc.vector.tensor_sub(out=hi[:, :, 0 : wo : wo - 1], in0=e0, in1=e1)

            yt = py.tile([n, 4, nout, wo], F32, tag="yt")
            if first:
                ts, bs, nint, oo = 1, 2, nout - 1, 1
                specials = [(1, 0, 0)]
            elif last:
                ts, bs, nint, oo = 0, 1, nout - 1, 0
                specials = [(nr - 1, nr - 2, nout - 1)]
            else:
                ts, bs, nint, oo = 0, 1, nout, 0
                specials = []

            for sb, src, sub in ((0, lo, False), (1, hi, False), (2, lo, True), (3, hi, True)):
                tv = src[:, ts : ts + 2 * nint - 1 : 2, :]
                bv = src[:, bs : bs + 2 * nint - 1 : 2, :]
                ov = yt[:, sb, oo : oo + nint, :]
                if sub:
                    nc.vector.tensor_sub(out=ov, in0=bv, in1=tv)
                else:
                    nc.vector.tensor_add(out=ov, in0=tv, in1=bv)
                for tt, bb, orow in specials:
                    tv1 = src[:, tt : tt + 1, :]
                    bv1 = src[:, bb : bb + 1, :]
                    ov1 = yt[:, sb, orow : orow + 1, :]
                    if sub:
                        nc.vector.tensor_sub(out=ov1, in0=bv1, in1=tv1)
                    else:
                        nc.vector.tensor_add(out=ov1, in0=tv1, in1=bv1)

            nc.scalar.dma_start(out=ya[:, :, i0 : i0 + nout, :], in_=yt[:])


def build_dwt_bass(n_img, h, w, ko=14, reps=1, out_engine="scalar", bufs=2, v2=False, v3=None):
    nc = bacc.Bacc("TRN2", target_bir_lowering=False, debug=False)
    x = nc.dram_tensor("x", [n_img, h, w], F32, kind="ExternalInput")
    y = nc.dram_tensor("y", [n_img, 4, h // 2 + 1, w // 2 + 1], F32, kind="ExternalOutput")
    with tile.TileContext(nc) as tc:
        for _ in range(reps):
            if v3 is not None:
                _emit_dwt_v3(tc, x.ap(), y.ap(), **v3)
            elif v2:
                _emit_dwt_v2(tc, x.ap(), y.ap(), ko, bufs=bufs)
            else:
                _emit_dwt(tc, x.ap(), y.ap(), ko, out_engine=out_engine, bufs=bufs)
    nc.compile()
    return nc


_NC_CACHE = {}


def _get_nc(n_img, h, w):
    key = (n_img, h, w)
    nc = _NC_CACHE.get(key)
    if nc is None:
        nc = _NC_CACHE[key] = build_dwt_bass(n_img, h, w)
    return nc


def kernel(x, _results_hook=None):
    x = np.ascontiguousarray(np.asarray(x), dtype=np.float32)
    b, c, h, w = x.shape
    n_total = b * c
    n_img = n_total // N_CORES
    nc = _get_nc(n_img, h, w)
    xf = x.reshape(n_total, h, w)
    in_maps = [{"x": xf[i * n_img : (i + 1) * n_img]} for i in range(N_CORES)]
    r = run_bass_kernel_spmd(nc, in_maps, list(range(N_CORES)))
    if _results_hook is not None:
        _results_hook(r)
    ho, wo = h // 2 + 1, w // 2 + 1
    out = np.concatenate([m["y"] for m in r.results], axis=0)
    out = out.reshape(b, c, 4, ho, wo).transpose(0, 2, 1, 3, 4).reshape(b, 4 * c, ho, wo)
    return np.ascontiguousarray(out)


# revision 17
# speedup vs baseline: 1.0338x; 1.0098x over previous
"""2D Haar DWT (single level, reflect-pad) Trainium2 Bass kernel.

Input  x: (16, 64, 256, 256) fp32
Output y: (16, 256, 129, 129) fp32, channel layout [ll(64C), lh, hl, hh].

Strategy: pure data parallel over 8 NeuronCores; each core gets 128 of the
1024 (batch, channel) images, one image per SBUF partition.  All butterfly
arithmetic runs along the free dimension:

  stage 1 (column butterfly, VectorE):
      lo[r, j] = 0.5*x[r, 2j-1] + 0.5*x[r, 2j]      (edges j=0, j=W/2 special)
      hi[r, j] = 0.5*x[r, 2j]   - 0.5*x[r, 2j-1]
  stage 2 (row butterfly, VectorE):
      ll[i] = lo[2i-1] + lo[2i]   lh[i] = hi[2i-1] + hi[2i]
      hl[i] = lo[2i]   - lo[2i-1] hh[i] = hi[2i]   - hi[2i-1]
      (edges i=0, i=H/2 special; the 0.5 prescale runs on ScalarE)

Output rows are produced in chunks of KO so DMAs stay in the multi-MB range.
"""

import numpy as np

import concourse.mybir as mybir
import concourse.tile as tile
from concourse import bacc
from concourse.bass_utils import run_bass_kernel_spmd

N_CORES = 8
F32 = mybir.dt.float32


def _chunk_list(h, ko):
    """(i0, n_out, r0, n_rows, first, last) chunks covering output rows 0..h/2."""
    hh = h // 2
    chunks = [(0, ko, 0, 2 * ko - 1, True, False)]
    i0 = ko
    while i0 < hh:
        ni = min(ko, hh - i0)
        r0 = 2 * i0 - 1
        if i0 + ni == hh:
            chunks.append((i0, ni + 1, r0, h - r0, False, True))
        else:
            chunks.append((i0, ni, r0, 2 * ni, False, False))
        i0 += ni
    return chunks


def _emit_dwt(tc, xa, ya, ko, out_engine="scalar", bufs=2, flat_out=True):
    nc = tc.nc
    n, h, w = xa.shape
    assert n == nc.NUM_PARTITIONS
    hh = h // 2
    wo = w // 2 + 1
    assert ko < hh

    chunks = _chunk_list(h, ko)
    # flat chunk-major output: per partition one contiguous 4*n_out*wo run per
    # chunk (28.9KB descriptors measure ~20% faster HBM writes than the
    # 4x7.2KB sb-strided runs of the [img][sb][i][j] layout)
    ya_flat = ya.rearrange("n a b c -> n (a b c)")
    off = 0

    with (
        tc.tile_pool(name="px", bufs=bufs) as px,
        tc.tile_pool(name="plh", bufs=2) as plh,
        tc.tile_pool(name="py", bufs=bufs) as py,
    ):
        for i0, nout, r0, nr, first, last in chunks:
            xt = px.tile([n, nr, w], F32, tag="xt")
            nc.sync.dma_start(out=xt[:], in_=xa[:, r0 : r0 + nr, :])
            nc.scalar.mul(xt[:], xt[:], 0.5)

            lo = plh.tile([n, nr, wo], F32, tag="lo")
            hi = plh.tile([n, nr, wo], F32, tag="hi")
            a = xt[:, :, 1 : w - 1 : 2]
            b = xt[:, :, 2:w:2]
            nc.vector.tensor_add(out=lo[:, :, 1 : wo - 1], in0=a, in1=b)
            nc.vector.tensor_sub(out=hi[:, :, 1 : wo - 1], in0=b, in1=a)
            # j=0 edge: cols (1, 0) of x -> lo = x0+x1, hi = x0-x1
            nc.vector.tensor_add(out=lo[:, :, 0:1], in0=xt[:, :, 0:1], in1=xt[:, :, 1:2])
            nc.vector.tensor_sub(out=hi[:, :, 0:1], in0=xt[:, :, 0:1], in1=xt[:, :, 1:2])
            # j=W/2 edge: cols (W-1, W-2) -> lo = x[W-2]+x[W-1], hi = x[W-2]-x[W-1]
            nc.vector.tensor_add(
                out=lo[:, :, wo - 1 : wo], in0=xt[:, :, w - 2 : w - 1], in1=xt[:, :, w - 1 : w]
            )
            nc.vector.tensor_sub(
                out=hi[:, :, wo - 1 : wo], in0=xt[:, :, w - 2 : w - 1], in1=xt[:, :, w - 1 : w]
            )

            yt = py.tile([n, 4, nout, wo], F32, tag="yt")
            if first:
                ts, bs, nint, oo = 1, 2, nout - 1, 1
                specials = [(1, 0, 0)]
            elif last:
                ts, bs, nint, oo = 0, 1, nout - 1, 0
                specials = [(nr - 1, nr - 2, nout - 1)]
            else:
                ts, bs, nint, oo = 0, 1, nout, 0
                specials = []

            for sb, src, sub in ((0, lo, False), (1, hi, False), (2, lo, True), (3, hi, True)):
                tv = src[:, ts : ts + 2 * nint - 1 : 2, :]
                bv = src[:, bs : bs + 2 * nint - 1 : 2, :]
                ov = yt[:, sb, oo : oo + nint, :]
                if sub:
                    nc.vector.tensor_sub(out=ov, in0=bv, in1=tv)
                else:
                    nc.vector.tensor_add(out=ov, in0=tv, in1=bv)
                for tt, bb, orow in specials:
                    tv1 = src[:, tt : tt + 1, :]
                    bv1 = src[:, bb : bb + 1, :]
                    ov1 = yt[:, sb, orow : orow + 1, :]
                    if sub:
                        nc.vector.tensor_sub(out=ov1, in0=bv1, in1=tv1)
                    else:
                        nc.vector.tensor_add(out=ov1, in0=tv1, in1=bv1)

            out_eng = {"scalar": nc.scalar, "sync": nc.sync, "gpsimd": nc.gpsimd}[out_engine]
            if flat_out:
                sz = 4 * nout * wo
                out_eng.dma_start(out=ya_flat[:, off : off + sz], in_=yt[:])
                off += sz
            else:
                out_eng.dma_start(out=ya[:, :, i0 : i0 + nout, :], in_=yt[:])


def _emit_dwt_v2(tc, xa, ya, ko, bufs=2):
    """Overlap-tuned variant: split in-DMA/prescale/stage1 into row halves,
    merge the two stage-1 edge columns into one strided op, and split the
    out-DMA across the scalar and gpsimd DGE rings."""
    nc = tc.nc
    n, h, w = xa.shape
    assert n == nc.NUM_PARTITIONS
    hh = h // 2
    wo = w // 2 + 1
    assert ko < hh

    chunks = [(0, ko, 0, 2 * ko - 1, True, False)]
    i0 = ko
    while i0 < hh:
        ni = min(ko, hh - i0)
        r0 = 2 * i0 - 1
        if i0 + ni == hh:
            chunks.append((i0, ni + 1, r0, h - r0, False, True))
        else:
            chunks.append((i0, ni, r0, 2 * ni, False, False))
        i0 += ni

    with (
        tc.tile_pool(name="px", bufs=bufs) as px,
        tc.tile_pool(name="plh", bufs=2) as plh,
        tc.tile_pool(name="py", bufs=bufs) as py,
    ):
        for i0, nout, r0, nr, first, last in chunks:
            xt = px.tile([n, nr, w], F32, tag="xt")
            lo = plh.tile([n, nr, wo], F32, tag="lo")
            hi = plh.tile([n, nr, wo], F32, tag="hi")
            h1 = nr // 2
            for lo_r, hi_r in ((0, h1), (h1, nr)):
                xh = xt[:, lo_r:hi_r, :]
                nc.sync.dma_start(out=xh, in_=xa[:, r0 + lo_r : r0 + hi_r, :])
                nc.scalar.mul(xh, xh, 0.5)
                a = xt[:, lo_r:hi_r, 1 : w - 1 : 2]
                b = xt[:, lo_r:hi_r, 2:w:2]
                nc.vector.tensor_add(out=lo[:, lo_r:hi_r, 1 : wo - 1], in0=a, in1=b)
                nc.vector.tensor_sub(out=hi[:, lo_r:hi_r, 1 : wo - 1], in0=b, in1=a)
                # merged edge op: cols {0,128} of lo/hi from x cols {0,254},{1,255}
                e0 = xt[:, lo_r:hi_r, 0 : w - 1 : w - 2]
                e1 = xt[:, lo_r:hi_r, 1:w : w - 2]
                nc.vector.tensor_add(out=lo[:, lo_r:hi_r, 0 : wo : wo - 1], in0=e0, in1=e1)
                nc.vector.tensor_sub(out=hi[:, lo_r:hi_r, 0 : wo : wo - 1], in0=e0, in1=e1)

            yt = py.tile([n, 4, nout, wo], F32, tag="yt")
            if first:
                ts, bs, nint, oo = 1, 2, nout - 1, 1
                specials = [(1, 0, 0)]
            elif last:
                ts, bs, nint, oo = 0, 1, nout - 1, 0
                specials = [(nr - 1, nr - 2, nout - 1)]
            else:
                ts, bs, nint, oo = 0, 1, nout, 0
                specials = []

            for sb, src, sub in ((0, lo, False), (1, hi, False), (2, lo, True), (3, hi, True)):
                tv = src[:, ts : ts + 2 * nint - 1 : 2, :]
                bv = src[:, bs : bs + 2 * nint - 1 : 2, :]
                ov = yt[:, sb, oo : oo + nint, :]
                if sub:
                    nc.vector.tensor_sub(out=ov, in0=bv, in1=tv)
                else:
                    nc.vector.tensor_add(out=ov, in0=tv, in1=bv)
                for tt, bb, orow in specials:
                    tv1 = src[:, tt : tt + 1, :]
                    bv1 = src[:, bb : bb + 1, :]
                    ov1 = yt[:, sb, orow : orow + 1, :]
                    if sub:
                        nc.vector.tensor_sub(out=ov1, in0=bv1, in1=tv1)
                    else:
                        nc.vector.tensor_add(out=ov1, in0=tv1, in1=bv1)
                if sb == 1:
                    nc.scalar.dma_start(out=ya[:, 0:2, i0 : i0 + nout, :], in_=yt[:, 0:2, :, :])
            nc.gpsimd.dma_start(out=ya[:, 2:4, i0 : i0 + nout, :], in_=yt[:, 2:4, :, :])


def _emit_dwt_v3(tc, xa, ya, ko=12, xbufs=3, ramp=True):
    """Deeper DMA queue variant: 3 input buffers (so two in-DMAs can be in
    flight beyond the chunk being computed), lo/hi fused into one tile to fit
    SBUF, optional small ramp-up chunks to shorten pipeline fill."""
    nc = tc.nc
    n, h, w = xa.shape
    assert n == nc.NUM_PARTITIONS
    hh = h // 2
    wo = w // 2 + 1

    # chunk output-row counts: optional small first chunks, then ko-sized,
    # remainder merged into the final chunk together with row i=hh.
    sizes = []
    rem = hh  # interior outputs 0..hh-1; i=hh rides with the last chunk
    if ramp and hh > 2 * ko:
        for s in (max(2, ko // 4), max(3, ko // 2)):
            sizes.append(s)
            rem -= s
    while rem > ko + 1:
        sizes.append(ko)
        rem -= ko
    sizes.append(rem)

    chunks = []
    i0 = 0
    for idx, sz in enumerate(sizes):
        first = idx == 0
        last = idx == len(sizes) - 1
        if first:
            chunks.append((0, sz, 0, 2 * sz - 1, True, False))
        elif last:
            r0 = 2 * i0 - 1
            chunks.append((i0, sz + 1, r0, h - r0, False, True))
        else:
            chunks.append((i0, sz, 2 * i0 - 1, 2 * sz, False, False))
        i0 += sz
    assert i0 == hh

    with (
        tc.tile_pool(name="px", bufs=xbufs) as px,
        tc.tile_pool(name="plh", bufs=2) as plh,
        tc.tile_pool(name="py", bufs=2) as py,
    ):
        for i0, nout, r0, nr, first, last in chunks:
            xt = px.tile([n, nr, w], F32, tag="xt")
            nc.sync.dma_start(out=xt[:], in_=xa[:, r0 : r0 + nr, :])
            nc.scalar.mul(xt[:], xt[:], 0.5)

            lh = plh.tile([n, 2, nr, wo], F32, tag="lh")
            lo = lh[:, 0]
            hi = lh[:, 1]
            a = xt[:, :, 1 : w - 1 : 2]
            b = xt[:, :, 2:w:2]
            nc.vector.tensor_add(out=lo[:, :, 1 : wo - 1], in0=a, in1=b)
            nc.vector.tensor_sub(out=hi[:, :, 1 : wo - 1], in0=b, in1=a)
            e0 = xt[:, :, 0 : w - 1 : w - 2]
            e1 = xt[:, :, 1:w : w - 2]
            nc.vector.tensor_add(out=lo[:, :, 0 : wo : wo - 1], in0=e0, in1=e1)
            nc.vector.tensor_sub(out=hi[:, :, 0 : wo : wo - 1], in0=e0, in1=e1)

            yt = py.tile([n, 4, nout, wo], F32, tag="yt")
            if first:
                ts, bs, nint, oo = 1, 2, nout - 1, 1
                specials = [(1, 0, 0)]
            elif last:
                ts, bs, nint, oo = 0, 1, nout - 1, 0
                specials = [(nr - 1, nr - 2, nout - 1)]
            else:
                ts, bs, nint, oo = 0, 1, nout, 0
                specials = []

            for sb, src, sub in ((0, lo, False), (1, hi, False), (2, lo, True), (3, hi, True)):
                tv = src[:, ts : ts + 2 * nint - 1 : 2, :]
                bv = src[:, bs : bs + 2 * nint - 1 : 2, :]
                ov = yt[:, sb, oo : oo + nint, :]
                if sub:
                    nc.vector.tensor_sub(out=ov, in0=bv, in1=tv)
                else:
                    nc.vector.tensor_add(out=ov, in0=tv, in1=bv)
                for tt, bb, orow in specials:
                    tv1 = src[:, tt : tt + 1, :]
                    bv1 = src[:, bb : bb + 1, :]
                    ov1 = yt[:, sb, orow : orow + 1, :]
                    if sub:
                        nc.vector.tensor_sub(out=ov1, in0=bv1, in1=tv1)
                    else:
                        nc.vector.tensor_add(out=ov1, in0=tv1, in1=bv1)

            nc.scalar.dma_start(out=ya[:, :, i0 : i0 + nout, :], in_=yt[:])


def build_dwt_bass(
    n_img, h, w, ko=14, reps=1, out_engine="scalar", bufs=2, v2=False, v3=None, flat_out=True
):
    nc = bacc.Bacc("TRN2", target_bir_lowering=False, debug=False)
    x = nc.dram_tensor("x", [n_img, h, w], F32, kind="ExternalInput")
    y = nc.dram_tensor("y", [n_img, 4, h // 2 + 1, w // 2 + 1], F32, kind="ExternalOutput")
    with tile.TileContext(nc) as tc:
        for _ in range(reps):
            if v3 is not None:
                _emit_dwt_v3(tc, x.ap(), y.ap(), **v3)
            elif v2:
                _emit_dwt_v2(tc, x.ap(), y.ap(), ko, bufs=bufs)
            else:
                _emit_dwt(tc, x.ap(), y.ap(), ko, out_engine=out_engine, bufs=bufs, flat_out=flat_out)
    nc.compile()
    return nc


_NC_CACHE = {}


def _get_nc(n_img, h, w):
    key = (n_img, h, w)
    nc = _NC_CACHE.get(key)
    if nc is None:
        nc = _NC_CACHE[key] = build_dwt_bass(n_img, h, w)
    return nc


def unscatter_flat(y_core, h, ko=14):
    """[n_img, 4*ho*wo] flat chunk-major device output -> [n_img, 4, ho, wo]."""
    ho = h // 2 + 1
    wo = ho
    n_img = y_core.shape[0]
    flat = y_core.reshape(n_img, 4 * ho * wo)
    out = np.empty((n_img, 4, ho, wo), np.float32)
    off = 0
    for i0, nout, _r0, _nr, _f, _l in _chunk_list(h, ko):
        sz = 4 * nout * wo
        out[:, :, i0 : i0 + nout, :] = flat[:, off : off + sz].reshape(n_img, 4, nout, wo)
        off += sz
    return out


def kernel(x, _results_hook=None):
    x = np.ascontiguousarray(np.asarray(x), dtype=np.float32)
    b, c, h, w = x.shape
    n_total = b * c
    n_img = n_total // N_CORES
    nc = _get_nc(n_img, h, w)
    xf = x.reshape(n_total, h, w)
    in_maps = [{"x": xf[i * n_img : (i + 1) * n_img]} for i in range(N_CORES)]
    r = run_bass_kernel_spmd(nc, in_maps, list(range(N_CORES)))
    if _results_hook is not None:
        _results_hook(r)
    ho, wo = h // 2 + 1, w // 2 + 1
    out = np.concatenate([unscatter_flat(m["y"], h) for m in r.results], axis=0)
    out = out.reshape(b, c, 4, ho, wo).transpose(0, 2, 1, 3, 4).reshape(b, 4 * c, ho, wo)
    return np.ascontiguousarray(out)


# revision 18
# speedup vs baseline: 1.0494x; 1.0151x over previous
"""2D Haar DWT (single level, reflect-pad) Trainium2 Bass kernel.

Input  x: (16, 64, 256, 256) fp32
Output y: (16, 256, 129, 129) fp32, channel layout [ll(64C), lh, hl, hh].

Strategy: pure data parallel over 8 NeuronCores; each core gets 128 of the
1024 (batch, channel) images, one image per SBUF partition.  All butterfly
arithmetic runs along the free dimension:

  stage 1 (column butterfly, VectorE):
      lo[r, j] = 0.5*x[r, 2j-1] + 0.5*x[r, 2j]      (edges j=0, j=W/2 special)
      hi[r, j] = 0.5*x[r, 2j]   - 0.5*x[r, 2j-1]
  stage 2 (row butterfly, VectorE):
      ll[i] = lo[2i-1] + lo[2i]   lh[i] = hi[2i-1] + hi[2i]
      hl[i] = lo[2i]   - lo[2i-1] hh[i] = hi[2i]   - hi[2i-1]
      (edges i=0, i=H/2 special; the 0.5 prescale runs on ScalarE)

Output rows are produced in chunks of KO so DMAs stay in the multi-MB range.
"""

import numpy as np

import concourse.mybir as mybir
import concourse.tile as tile
from concourse import bacc
from concourse.bass_utils import run_bass_kernel_spmd

N_CORES = 8
F32 = mybir.dt.float32
KO_DEFAULT = 15


def _chunk_list(h, ko):
    """(i0, n_out, r0, n_rows, first, last) chunks covering output rows 0..h/2."""
    hh = h // 2
    chunks = [(0, ko, 0, 2 * ko - 1, True, False)]
    i0 = ko
    while i0 < hh:
        ni = min(ko, hh - i0)
        r0 = 2 * i0 - 1
        if i0 + ni == hh:
            chunks.append((i0, ni + 1, r0, h - r0, False, True))
        else:
            chunks.append((i0, ni, r0, 2 * ni, False, False))
        i0 += ni
    return chunks


def _emit_dwt(tc, xa, ya, ko, out_engine="scalar", bufs=2, flat_out=True):
    nc = tc.nc
    n, h, w = xa.shape
    assert n == nc.NUM_PARTITIONS
    hh = h // 2
    wo = w // 2 + 1
    assert ko < hh

    chunks = _chunk_list(h, ko)
    # flat chunk-major output: per partition one contiguous 4*n_out*wo run per
    # chunk (28.9KB descriptors measure ~20% faster HBM writes than the
    # 4x7.2KB sb-strided runs of the [img][sb][i][j] layout)
    ya_flat = ya.rearrange("n a b c -> n (a b c)")
    off = 0

    with (
        tc.tile_pool(name="px", bufs=bufs) as px,
        tc.tile_pool(name="plh", bufs=2) as plh,
        tc.tile_pool(name="py", bufs=bufs) as py,
    ):
        for i0, nout, r0, nr, first, last in chunks:
            xt = px.tile([n, nr, w], F32, tag="xt")
            nc.sync.dma_start(out=xt[:], in_=xa[:, r0 : r0 + nr, :])
            nc.scalar.mul(xt[:], xt[:], 0.5)

            lo = plh.tile([n, nr, wo], F32, tag="lo")
            hi = plh.tile([n, nr, wo], F32, tag="hi")
            a = xt[:, :, 1 : w - 1 : 2]
            b = xt[:, :, 2:w:2]
            nc.vector.tensor_add(out=lo[:, :, 1 : wo - 1], in0=a, in1=b)
            nc.vector.tensor_sub(out=hi[:, :, 1 : wo - 1], in0=b, in1=a)
            # j=0 edge: cols (1, 0) of x -> lo = x0+x1, hi = x0-x1
            nc.vector.tensor_add(out=lo[:, :, 0:1], in0=xt[:, :, 0:1], in1=xt[:, :, 1:2])
            nc.vector.tensor_sub(out=hi[:, :, 0:1], in0=xt[:, :, 0:1], in1=xt[:, :, 1:2])
            # j=W/2 edge: cols (W-1, W-2) -> lo = x[W-2]+x[W-1], hi = x[W-2]-x[W-1]
            nc.vector.tensor_add(
                out=lo[:, :, wo - 1 : wo], in0=xt[:, :, w - 2 : w - 1], in1=xt[:, :, w - 1 : w]
            )
            nc.vector.tensor_sub(
                out=hi[:, :, wo - 1 : wo], in0=xt[:, :, w - 2 : w - 1], in1=xt[:, :, w - 1 : w]
            )

            yt = py.tile([n, 4, nout, wo], F32, tag="yt")
            if first:
                ts, bs, nint, oo = 1, 2, nout - 1, 1
                specials = [(1, 0, 0)]
            elif last:
                ts, bs, nint, oo = 0, 1, nout - 1, 0
                specials = [(nr - 1, nr - 2, nout - 1)]
            else:
                ts, bs, nint, oo = 0, 1, nout, 0
                specials = []

            for sb, src, sub in ((0, lo, False), (1, hi, False), (2, lo, True), (3, hi, True)):
                tv = src[:, ts : ts + 2 * nint - 1 : 2, :]
                bv = src[:, bs : bs + 2 * nint - 1 : 2, :]
                ov = yt[:, sb, oo : oo + nint, :]
                if sub:
                    nc.vector.tensor_sub(out=ov, in0=bv, in1=tv)
                else:
                    nc.vector.tensor_add(out=ov, in0=tv, in1=bv)
                for tt, bb, orow in specials:
                    tv1 = src[:, tt : tt + 1, :]
                    bv1 = src[:, bb : bb + 1, :]
                    ov1 = yt[:, sb, orow : orow + 1, :]
                    if sub:
                        nc.vector.tensor_sub(out=ov1, in0=bv1, in1=tv1)
                    else:
                        nc.vector.tensor_add(out=ov1, in0=tv1, in1=bv1)

            out_eng = {"scalar": nc.scalar, "sync": nc.sync, "gpsimd": nc.gpsimd}[out_engine]
            if flat_out:
                sz = 4 * nout * wo
                out_eng.dma_start(out=ya_flat[:, off : off + sz], in_=yt[:])
                off += sz
            else:
                out_eng.dma_start(out=ya[:, :, i0 : i0 + nout, :], in_=yt[:])


def _emit_dwt_v2(tc, xa, ya, ko, bufs=2):
    """Overlap-tuned variant: split in-DMA/prescale/stage1 into row halves,
    merge the two stage-1 edge columns into one strided op, and split the
    out-DMA across the scalar and gpsimd DGE rings."""
    nc = tc.nc
    n, h, w = xa.shape
    assert n == nc.NUM_PARTITIONS
    hh = h // 2
    wo = w // 2 + 1
    assert ko < hh

    chunks = [(0, ko, 0, 2 * ko - 1, True, False)]
    i0 = ko
    while i0 < hh:
        ni = min(ko, hh - i0)
        r0 = 2 * i0 - 1
        if i0 + ni == hh:
            chunks.append((i0, ni + 1, r0, h - r0, False, True))
        else:
            chunks.append((i0, ni, r0, 2 * ni, False, False))
        i0 += ni

    with (
        tc.tile_pool(name="px", bufs=bufs) as px,
        tc.tile_pool(name="plh", bufs=2) as plh,
        tc.tile_pool(name="py", bufs=bufs) as py,
    ):
        for i0, nout, r0, nr, first, last in chunks:
            xt = px.tile([n, nr, w], F32, tag="xt")
            lo = plh.tile([n, nr, wo], F32, tag="lo")
            hi = plh.tile([n, nr, wo], F32, tag="hi")
            h1 = nr // 2
            for lo_r, hi_r in ((0, h1), (h1, nr)):
                xh = xt[:, lo_r:hi_r, :]
                nc.sync.dma_start(out=xh, in_=xa[:, r0 + lo_r : r0 + hi_r, :])
                nc.scalar.mul(xh, xh, 0.5)
                a = xt[:, lo_r:hi_r, 1 : w - 1 : 2]
                b = xt[:, lo_r:hi_r, 2:w:2]
                nc.vector.tensor_add(out=lo[:, lo_r:hi_r, 1 : wo - 1], in0=a, in1=b)
                nc.vector.tensor_sub(out=hi[:, lo_r:hi_r, 1 : wo - 1], in0=b, in1=a)
                # merged edge op: cols {0,128} of lo/hi from x cols {0,254},{1,255}
                e0 = xt[:, lo_r:hi_r, 0 : w - 1 : w - 2]
                e1 = xt[:, lo_r:hi_r, 1:w : w - 2]
                nc.vector.tensor_add(out=lo[:, lo_r:hi_r, 0 : wo : wo - 1], in0=e0, in1=e1)
                nc.vector.tensor_sub(out=hi[:, lo_r:hi_r, 0 : wo : wo - 1], in0=e0, in1=e1)

            yt = py.tile([n, 4, nout, wo], F32, tag="yt")
            if first:
                ts, bs, nint, oo = 1, 2, nout - 1, 1
                specials = [(1, 0, 0)]
            elif last:
                ts, bs, nint, oo = 0, 1, nout - 1, 0
                specials = [(nr - 1, nr - 2, nout - 1)]
            else:
                ts, bs, nint, oo = 0, 1, nout, 0
                specials = []

            for sb, src, sub in ((0, lo, False), (1, hi, False), (2, lo, True), (3, hi, True)):
                tv = src[:, ts : ts + 2 * nint - 1 : 2, :]
                bv = src[:, bs : bs + 2 * nint - 1 : 2, :]
                ov = yt[:, sb, oo : oo + nint, :]
                if sub:
                    nc.vector.tensor_sub(out=ov, in0=bv, in1=tv)
                else:
                    nc.vector.tensor_add(out=ov, in0=tv, in1=bv)
                for tt, bb, orow in specials:
                    tv1 = src[:, tt : tt + 1, :]
                    bv1 = src[:, bb : bb + 1, :]
                    ov1 = yt[:, sb, orow : orow + 1, :]
                    if sub:
                        nc.vector.tensor_sub(out=ov1, in0=bv1, in1=tv1)
                    else:
                        nc.vector.tensor_add(out=ov1, in0=tv1, in1=bv1)
                if sb == 1:
                    nc.scalar.dma_start(out=ya[:, 0:2, i0 : i0 + nout, :], in_=yt[:, 0:2, :, :])
            nc.gpsimd.dma_start(out=ya[:, 2:4, i0 : i0 + nout, :], in_=yt[:, 2:4, :, :])


def _emit_dwt_v3(tc, xa, ya, ko=12, xbufs=3, ramp=True):
    """Deeper DMA queue variant: 3 input buffers (so two in-DMAs can be in
    flight beyond the chunk being computed), lo/hi fused into one tile to fit
    SBUF, optional small ramp-up chunks to shorten pipeline fill."""
    nc = tc.nc
    n, h, w = xa.shape
    assert n == nc.NUM_PARTITIONS
    hh = h // 2
    wo = w // 2 + 1

    # chunk output-row counts: optional small first chunks, then ko-sized,
    # remainder merged into the final chunk together with row i=hh.
    sizes = []
    rem = hh  # interior outputs 0..hh-1; i=hh rides with the last chunk
    if ramp and hh > 2 * ko:
        for s in (max(2, ko // 4), max(3, ko // 2)):
            sizes.append(s)
            rem -= s
    while rem > ko + 1:
        sizes.append(ko)
        rem -= ko
    sizes.append(rem)

    chunks = []
    i0 = 0
    for idx, sz in enumerate(sizes):
        first = idx == 0
        last = idx == len(sizes) - 1
        if first:
            chunks.append((0, sz, 0, 2 * sz - 1, True, False))
        elif last:
            r0 = 2 * i0 - 1
            chunks.append((i0, sz + 1, r0, h - r0, False, True))
        else:
            chunks.append((i0, sz, 2 * i0 - 1, 2 * sz, False, False))
        i0 += sz
    assert i0 == hh

    with (
        tc.tile_pool(name="px", bufs=xbufs) as px,
        tc.tile_pool(name="plh", bufs=2) as plh,
        tc.tile_pool(name="py", bufs=2) as py,
    ):
        for i0, nout, r0, nr, first, last in chunks:
            xt = px.tile([n, nr, w], F32, tag="xt")
            nc.sync.dma_start(out=xt[:], in_=xa[:, r0 : r0 + nr, :])
            nc.scalar.mul(xt[:], xt[:], 0.5)

            lh = plh.tile([n, 2, nr, wo], F32, tag="lh")
            lo = lh[:, 0]
            hi = lh[:, 1]
            a = xt[:, :, 1 : w - 1 : 2]
            b = xt[:, :, 2:w:2]
            nc.vector.tensor_add(out=lo[:, :, 1 : wo - 1], in0=a, in1=b)
            nc.vector.tensor_sub(out=hi[:, :, 1 : wo - 1], in0=b, in1=a)
            e0 = xt[:, :, 0 : w - 1 : w - 2]
            e1 = xt[:, :, 1:w : w - 2]
            nc.vector.tensor_add(out=lo[:, :, 0 : wo : wo - 1], in0=e0, in1=e1)
            nc.vector.tensor_sub(out=hi[:, :, 0 : wo : wo - 1], in0=e0, in1=e1)

            yt = py.tile([n, 4, nout, wo], F32, tag="yt")
            if first:
                ts, bs, nint, oo = 1, 2, nout - 1, 1
                specials = [(1, 0, 0)]
            elif last:
                ts, bs, nint, oo = 0, 1, nout - 1, 0
                specials = [(nr - 1, nr - 2, nout - 1)]
            else:
                ts, bs, nint, oo = 0, 1, nout, 0
                specials = []

            for sb, src, sub in ((0, lo, False), (1, hi, False), (2, lo, True), (3, hi, True)):
                tv = src[:, ts : ts + 2 * nint - 1 : 2, :]
                bv = src[:, bs : bs + 2 * nint - 1 : 2, :]
                ov = yt[:, sb, oo : oo + nint, :]
                if sub:
                    nc.vector.tensor_sub(out=ov, in0=bv, in1=tv)
                else:
                    nc.vector.tensor_add(out=ov, in0=tv, in1=bv)
                for tt, bb, orow in specials:
                    tv1 = src[:, tt : tt + 1, :]
                    bv1 = src[:, bb : bb + 1, :]
                    ov1 = yt[:, sb, orow : orow + 1, :]
                    if sub:
                        nc.vector.tensor_sub(out=ov1, in0=bv1, in1=tv1)
                    else:
                        nc.vector.tensor_add(out=ov1, in0=tv1, in1=bv1)

            nc.scalar.dma_start(out=ya[:, :, i0 : i0 + nout, :], in_=yt[:])


def build_dwt_bass(
    n_img, h, w, ko=KO_DEFAULT, reps=1, out_engine="scalar", bufs=2, v2=False, v3=None, flat_out=True
):
    nc = bacc.Bacc("TRN2", target_bir_lowering=False, debug=False)
    x = nc.dram_tensor("x", [n_img, h, w], F32, kind="ExternalInput")
    y = nc.dram_tensor("y", [n_img, 4, h // 2 + 1, w // 2 + 1], F32, kind="ExternalOutput")
    with tile.TileContext(nc) as tc:
        for _ in range(reps):
            if v3 is not None:
                _emit_dwt_v3(tc, x.ap(), y.ap(), **v3)
            elif v2:
                _emit_dwt_v2(tc, x.ap(), y.ap(), ko, bufs=bufs)
            else:
                _emit_dwt(tc, x.ap(), y.ap(), ko, out_engine=out_engine, bufs=bufs, flat_out=flat_out)
    nc.compile()
    return nc


_NC_CACHE = {}


def _get_nc(n_img, h, w):
    key = (n_img, h, w)
    nc = _NC_CACHE.get(key)
    if nc is None:
        nc = _NC_CACHE[key] = build_dwt_bass(n_img, h, w)
    return nc


def unscatter_flat(y_core, h, ko=KO_DEFAULT):
    """[n_img, 4*ho*wo] flat chunk-major device output -> [n_img, 4, ho, wo]."""
    ho = h // 2 + 1
    wo = ho
    n_img = y_core.shape[0]
    flat = y_core.reshape(n_img, 4 * ho * wo)
    out = np.empty((n_img, 4, ho, wo), np.float32)
    off = 0
    for i0, nout, _r0, _nr, _f, _l in _chunk_list(h, ko):
        sz = 4 * nout * wo
        out[:, :, i0 : i0 + nout, :] = flat[:, off : off + sz].reshape(n_img, 4, nout, wo)
        off += sz
    return out


def kernel(x, _results_hook=None):
    x = np.ascontiguousarray(np.asarray(x), dtype=np.float32)
    b, c, h, w = x.shape
    n_total = b * c
    n_img = n_total // N_CORES
    nc = _get_nc(n_img, h, w)
    xf = x.reshape(n_total, h, w)
    in_maps = [{"x": xf[i * n_img : (i + 1) * n_img]} for i in range(N_CORES)]
    r = run_bass_kernel_spmd(nc, in_maps, list(range(N_CORES)))
    if _results_hook is not None:
        _results_hook(r)
    ho, wo = h // 2 + 1, w // 2 + 1
    out = np.concatenate([unscatter_flat(m["y"], h) for m in r.results], axis=0)
    out = out.reshape(b, c, 4, ho, wo).transpose(0, 2, 1, 3, 4).reshape(b, 4 * c, ho, wo)
    return np.ascontiguousarray(out)
